# revision 21
# baseline (speedup 1.0000x reference)
"""DGCNN (3x DynamicEdgeConv + MLP head) on 8 Trainium2 NeuronCores.

Data-parallel over the cloud/batch axis: each core processes 16 of the 128
point clouds. All compute per cloud happens on-chip:

  - kNN: scores S[i,j] = 2<x_i,x_j> - |x_j|^2 (argmax == nearest) via PE
    matmuls into PSUM, then DVE max8/max_index directly on PSUM.
  - Edge MLP factorization: e @ W1 = x_i @ (W1a-W1b) + x_j @ W1b, so only two
    per-point matmuls (U, V) + a free-dim gather of V columns (GPSIMD
    ap_gather) are needed; VG + U_broadcast runs as two accumulating identity
    matmuls, ReLU on ACT, second linear on PE, neighbor-max on GPSIMD.
  - Conv biases b2 are folded into downstream weights on the host (distances
    are shift-invariant; max-agg commutes with per-channel shifts).
  - Final MLP in feature-transposed layout (264 = 128+128+8 row chunks);
    log_softmax batched once per core at the end.

Edge order: edge n (n = 0..4095 per cloud) maps to point i(n) = 16*(n//64) +
n%16 and slot s(n) = (n%64)//16.  This makes the ap_gather index list land in
its "wrapped 16-partition" layout with each point's 4 indices one contiguous
8-byte run, writable by plain DMAs from the max_index output.
"""
import sys
from contextlib import ExitStack

import numpy as np

sys.path.insert(0, "/opt/trn_rl_repo")

import concourse.bass as bass
import concourse.bacc as bacc
import concourse.mybir as mybir
from concourse import tile
from concourse import bass_utils

F32 = mybir.dt.float32
I16 = mybir.dt.int16
U32 = mybir.dt.uint32
AF = mybir.ActivationFunctionType
OP = mybir.AluOpType

P = 1024          # points per cloud
B = 128           # clouds total
NCORES = 8
NCL = B // NCORES  # clouds per core
NT = P // 128      # row tiles per cloud
HID = 264
LAYERS = [(1, 32), (32, 32), (32, 64)]  # (d_in, c_out)
KCH = [(0, 128), (128, 256), (256, 264)]  # 264-dim chunking


def build_program(ncl=NCL):
    nc = bacc.Bacc("TRN2", target_bir_lowering=False, debug=False)
    dram = {}

    def din(name, shape, dtype=F32):
        dram[name] = nc.dram_tensor(name, list(shape), dtype, kind="ExternalInput")
        return dram[name]

    din("xT", (1, ncl * P))
    for l, (d, c) in enumerate(LAYERS, 1):
        din(f"A{l}", (d, c))
        din(f"B{l}", (d, c))
        din(f"ub{l}", (c, 1))
        din(f"W2_{l}", (c, c))
        din(f"I{l}", (c, c))
    din("mW1a", (32, HID))
    din("mW1b", (32, HID))
    din("mW1c", (64, HID))
    din("mb1", (HID, 1))
    din("mW2", (HID, HID))
    din("mb2", (HID, 1))
    din("mW3", (HID, HID))
    din("mb3", (HID, 1))
    din("mW4", (HID, 2))
    din("mb4", (1, 2))
    out_d = nc.dram_tensor("out", [ncl * P, 2], F32, kind="ExternalOutput")

    with tile.TileContext(nc) as tc, ExitStack() as ctx:
        wp = ctx.enter_context(tc.tile_pool(name="w", bufs=1))
        sb = ctx.enter_context(tc.tile_pool(name="sb", bufs=2))
        ps = ctx.enter_context(tc.tile_pool(name="ps", bufs=1, space="PSUM"))

        W = {}

        def load_w(name, shape):
            t = wp.tile(list(shape), F32, tag=name, bufs=1)
            nc.sync.dma_start(t[:], dram[name][0:shape[0], 0:shape[1]])
            W[name] = t
            return t

        for l, (d, c) in enumerate(LAYERS, 1):
            load_w(f"A{l}", (d, c))
            load_w(f"B{l}", (d, c))
            load_w(f"ub{l}", (c, 1))
            load_w(f"W2_{l}", (c, c))
            load_w(f"I{l}", (c, c))
        load_w("mW1a", (32, HID))
        load_w("mW1b", (32, HID))
        load_w("mW1c", (64, HID))
        load_w("mb4", (1, 2))
        # chunked 264-row tensors
        for nm in ["mW2", "mW3"]:
            for ki, (k0, k1) in enumerate(KCH):
                t = wp.tile([k1 - k0, HID], F32, tag=f"{nm}k{ki}", bufs=1)
                nc.sync.dma_start(t[:], dram[nm][k0:k1, :])
                W[f"{nm}k{ki}"] = t
        for ki, (k0, k1) in enumerate(KCH):
            t = wp.tile([k1 - k0, 2], F32, tag=f"mW4k{ki}", bufs=1)
            nc.sync.dma_start(t[:], dram["mW4"][k0:k1, :])
            W[f"mW4k{ki}"] = t
        for nm in ["mb1", "mb2", "mb3"]:
            for ki, (k0, k1) in enumerate(KCH):
                t = wp.tile([k1 - k0, 1], F32, tag=f"{nm}k{ki}", bufs=1)
                nc.sync.dma_start(t[:], dram[nm][k0:k1, :])
                W[f"{nm}k{ki}"] = t

        ones_d = wp.tile([32, 1], F32, tag="ones_d", bufs=1)
        nc.gpsimd.memset(ones_d[:], 1.0)
        ones_r = wp.tile([1, P], F32, tag="ones_r", bufs=1)
        nc.gpsimd.memset(ones_r[:], 1.0)

        OT = wp.tile([128, 2 * NT * ncl], F32, tag="ot", bufs=1)
        OUT = wp.tile([128, 2 * NT * ncl], F32, tag="outsb", bufs=1)

        def conv_layer(l, d, c, Xin, Fout):
            """Xin: [d, P] AP (pre-b2 features); Fout: [c, P] AP destination.
            All APs based at partition 0."""
            A, Bw, ub = W[f"A{l}"], W[f"B{l}"], W[f"ub{l}"]
            W2, Ic = W[f"W2_{l}"], W[f"I{l}"]
            rhx = sb.tile([d, P], F32, tag="rhx", bufs=2)
            nc.scalar.mul(rhx[:], Xin, 2.0)
            xsq = sb.tile([d, P], F32, tag="xsq", bufs=1)
            nc.scalar.activation(xsq[:], Xin, AF.Square)
            nsq = sb.tile([1, P], F32, tag="nsq", bufs=2)
            for jc in range(2):
                sqp_t = ps.tile([64, 512], F32, tag="sp1", bufs=2)
                sqp = sqp_t[0:1, :]
                nc.tensor.matmul(sqp[:], ones_d[0:d, :], xsq[:, jc * 512:(jc + 1) * 512],
                                 start=True, stop=True)
                nc.scalar.mul(nsq[0:1, jc * 512:(jc + 1) * 512], sqp[:], -1.0)
            # U = A^T X + ub, V = B^T X
            U = sb.tile([c, P], F32, tag="U", bufs=2)
            V = sb.tile([c, P], F32, tag="V", bufs=2)
            for jc in range(2):
                up = ps.tile([64, 512], F32, tag="sp1", bufs=2)
                nc.tensor.matmul(up[0:c, :], A[:], Xin[:, jc * 512:(jc + 1) * 512],
                                 start=True, stop=True)
                nc.scalar.activation(U[:, jc * 512:(jc + 1) * 512], up[0:c, :],
                                     AF.Identity, bias=ub[:])
                vp = ps.tile([64, 512], F32, tag="sp1", bufs=2)
                nc.tensor.matmul(vp[0:c, :], Bw[:], Xin[:, jc * 512:(jc + 1) * 512],
                                 start=True, stop=True)
                nc.scalar.copy(V[:, jc * 512:(jc + 1) * 512], vp[0:c, :])
            # kNN per row tile -> wrapped int16 index list
            widx = sb.tile([64, 4 * P // 16], I16, tag="widx", bufs=2)
            idx16 = sb.tile([128, 4 * NT], I16, tag="idx16", bufs=2)
            for t in range(NT):
                Sp = ps.tile([128, P], F32, tag="Sps", bufs=2)
                for jc in range(2):
                    nc.tensor.matmul(Sp[:, jc * 512:(jc + 1) * 512],
                                     Xin[:, t * 128:(t + 1) * 128],
                                     rhx[:, jc * 512:(jc + 1) * 512],
                                     start=True, stop=False)
                    nc.tensor.matmul(Sp[:, jc * 512:(jc + 1) * 512],
                                     ones_r[:, t * 128:(t + 1) * 128],
                                     nsq[:, jc * 512:(jc + 1) * 512],
                                     start=False, stop=True)
                Ssb = sb.tile([128, P], F32, tag="Ssb", bufs=2)
                nc.scalar.copy(Ssb[:], Sp[:])
                mx = sb.tile([128, 8], F32, tag="mx", bufs=3)
                ix = sb.tile([128, 8], U32, tag="ix", bufs=3)
                nc.vector.max(mx[:], Ssb[:])
                nc.vector.max_index(ix[:], mx[:], Ssb[:])
                nc.vector.tensor_copy(idx16[:, t * 4:(t + 1) * 4], ix[:, 0:4])
            # wrap: widx[p, 32t + 4w + s] = idx16[16w + p, 4t + s]; one DMA per w
            # (src [16, (t,s)] contiguous; dst runs of 4 i16, middle-dim stride)
            iview = idx16[:].rearrange("(w p) (t s) -> p w t s", p=16, s=4)
            wview = widx[0:16, :].rearrange("p (t w s) -> p w t s", w=8, s=4)
            for w in range(NT):
                nc.sync.dma_start(wview[:, w], iview[:, w])
            for g in range(1, c // 16):
                nc.sync.dma_start(widx[16 * g:16 * (g + 1), :], widx[0:16, :])
            # gather V columns; VG column n -> point i(n)=16*(n//64)+n%16, slot s(n)
            VG = sb.tile([c, 4 * P], F32, tag="VG", bufs=2)
            nc.gpsimd.ap_gather(VG[:], V[:], widx[0:c, :], channels=c,
                                num_elems=P, d=1, num_idxs=4 * P)
            # pre = VG + U[:, i(n)] via two accumulating identity matmuls
            R = sb.tile([c, 4 * P], F32, tag="R", bufs=1)
            H = sb.tile([c, 4 * P], F32, tag="H", bufs=1)
            uview = U[:].rearrange("c (A p) -> c A p", p=16)
            for ch in range(8):
                pp = ps.tile([64, 512], F32, tag="sp1", bufs=2)
                nc.tensor.matmul(pp[0:c, :], Ic[:], VG[:, ch * 512:(ch + 1) * 512],
                                 start=True, stop=False)
                # rhs: U[:, 128ch + 16a + p] repeated over s: dims [a:8][s:4(x0)][p:16]
                usl = uview[:, ch * 8:(ch + 1) * 8, :]
                ub_ap = bass.AP(usl.tensor, usl.offset,
                                [list(usl.ap[0]), list(usl.ap[1]), [0, 4],
                                 list(usl.ap[2])])
                nc.tensor.matmul(pp[0:c, :], Ic[:], ub_ap, start=False, stop=True)
                nc.scalar.activation(R[:, ch * 512:(ch + 1) * 512], pp[0:c, :], AF.Relu)
                hp = ps.tile([64, 512], F32, tag="sp1", bufs=2)
                nc.tensor.matmul(hp[0:c, :], W2[:], R[:, ch * 512:(ch + 1) * 512],
                                 start=True, stop=True)
                nc.scalar.copy(H[:, ch * 512:(ch + 1) * 512], hp[0:c, :])
            # neighbor max over the 4 slots: H col n, s-slices are [A:64][p:16]
            # (walrus rejects generic vector ops on GPSIMD -> DVE 2-op tree)
            hv = H[:].rearrange("c (A s p) -> c A s p", s=4, p=16)
            T1 = sb.tile([c, 2 * P], F32, tag="T1", bufs=1)
            t1v = T1[:].rearrange("c (A s p) -> c A s p", s=2, p=16)
            fv = Fout.rearrange("c (A p) -> c A p", p=16)
            nc.vector.tensor_tensor(t1v, hv[:, :, 0:2, :], hv[:, :, 2:4, :], OP.max)
            nc.vector.tensor_tensor(fv, t1v[:, :, 0, :], t1v[:, :, 1, :], OP.max)

        def final_mlp(b, X1, X2, X3):
            def dense(lhs_chunks, rhs_chunks, bias_key, out_tag):
                outs = []
                for mi, (m0, m1) in enumerate(KCH):
                    mcs = m1 - m0
                    hp = ps.tile([mcs, P], F32, tag="hmlp", bufs=1)
                    nk = len(lhs_chunks)
                    for jc in range(2):
                        for ki in range(nk):
                            nc.tensor.matmul(
                                hp[:, jc * 512:(jc + 1) * 512],
                                lhs_chunks[ki][:, m0:m1],
                                rhs_chunks[ki][:, jc * 512:(jc + 1) * 512],
                                start=(ki == 0), stop=(ki == nk - 1))
                    ho = sb.tile([mcs, P], F32, tag=f"{out_tag}{mi}", bufs=1)
                    nc.scalar.activation(ho[:], hp[:], AF.Relu,
                                         bias=W[f"{bias_key}k{mi}"][:])
                    outs.append(ho)
                return outs

            h1 = dense([W["mW1a"], W["mW1b"], W["mW1c"]], [X1, X2, X3],
                       "mb1", "h1c")
            h2 = dense([W["mW2k0"], W["mW2k1"], W["mW2k2"]], h1, "mb2", "h2c")
            h3 = dense([W["mW3k0"], W["mW3k1"], W["mW3k2"]], h2, "mb3", "h3c")
            lg = ps.tile([128, 2 * NT], F32, tag="sp1", bufs=2)
            for t in range(NT):
                o = lg[:, t * 2:(t + 1) * 2]
                for ki in range(3):
                    nc.tensor.matmul(o, h3[ki][:, t * 128:(t + 1) * 128],
                                     W[f"mW4k{ki}"][:], start=(ki == 0), stop=False)
                nc.tensor.matmul(o, ones_r[:, t * 128:(t + 1) * 128], W["mb4"][:],
                                 start=False, stop=True)
            nc.scalar.copy(OT[:, b * 2 * NT:(b + 1) * 2 * NT], lg[:])

        for b in range(ncl):
            xin = sb.tile([1, P], F32, tag="xin", bufs=2)
            nc.sync.dma_start(xin[:], dram["xT"][0:1, b * P:(b + 1) * P])
            X1 = sb.tile([32, P], F32, tag="X1", bufs=2)
            X2 = sb.tile([32, P], F32, tag="X2", bufs=2)
            X3 = sb.tile([64, P], F32, tag="X3", bufs=2)
            conv_layer(1, 1, 32, xin[:], X1[:])
            conv_layer(2, 32, 32, X1[:], X2[:])
            conv_layer(3, 32, 64, X2[:], X3[:])
            final_mlp(b, X1[:], X2[:], X3[:])

        # batched log_softmax over class pairs: lse = max + softplus(-|o0-o1|)
        o0 = OT[:, 0::2]
        o1 = OT[:, 1::2]
        n = NT * ncl
        m_ = wp.tile([128, n], F32, tag="lsm", bufs=1)
        dd = wp.tile([128, n], F32, tag="lsd", bufs=1)
        nc.vector.tensor_tensor(m_[:], o0, o1, OP.max)
        nc.vector.tensor_tensor(dd[:], o0, o1, OP.subtract)
        aa = wp.tile([128, n], F32, tag="lsa", bufs=1)
        nc.scalar.activation(aa[:], dd[:], AF.Abs)
        ee = wp.tile([128, n], F32, tag="lsee", bufs=1)
        nc.scalar.activation(ee[:], aa[:], AF.Exp, scale=-1.0)
        sp = wp.tile([128, n], F32, tag="lssp", bufs=1)
        nc.scalar.activation(sp[:], ee[:], AF.Ln, bias=1.0)
        lse = wp.tile([128, n], F32, tag="lse", bufs=1)
        nc.vector.tensor_tensor(lse[:], m_[:], sp[:], OP.add)
        nc.vector.tensor_tensor(OUT[:, 0::2], o0, lse[:], OP.subtract)
        nc.vector.tensor_tensor(OUT[:, 1::2], o1, lse[:], OP.subtract)
        for bt in range(ncl * NT):
            nc.sync.dma_start(out_d[bt * 128:(bt + 1) * 128, :],
                              OUT[:, bt * 2:bt * 2 + 2])

    nc.compile()
    return nc


def prep_weights(inp):
    """Host-side weight transforms (b2 folding + edge-MLP factorization)."""
    w = {}
    prev_b2 = None
    for l, pre in enumerate(["c1", "c2", "c3"], start=1):
        W1 = np.asarray(inp[f"{pre}W1"], np.float32)
        b1 = np.asarray(inp[f"{pre}b1"], np.float32)
        W2 = np.asarray(inp[f"{pre}W2"], np.float32)
        b2 = np.asarray(inp[f"{pre}b2"], np.float32)
        d = W1.shape[0] // 2
        W1a, W1b = W1[:d], W1[d:]
        A = (W1a - W1b).astype(np.float32)
        Bw = W1b.astype(np.float32)
        ub = b1.copy()
        if prev_b2 is not None:
            ub = ub + prev_b2 @ (A + Bw)
        c = W1.shape[1]
        w[f"A{l}"] = A
        w[f"B{l}"] = Bw
        w[f"ub{l}"] = ub.astype(np.float32).reshape(c, 1)
        w[f"W2_{l}"] = W2
        w[f"I{l}"] = np.eye(c, dtype=np.float32)
        prev_b2 = b2
    beta = np.concatenate([np.asarray(inp["c1b2"]), np.asarray(inp["c2b2"]),
                           np.asarray(inp["c3b2"])]).astype(np.float32)
    mW1 = np.asarray(inp["mW1"], np.float32)
    w["mW1a"] = mW1[0:32]
    w["mW1b"] = mW1[32:64]
    w["mW1c"] = mW1[64:128]
    w["mb1"] = (np.asarray(inp["mb1"], np.float32) + beta @ mW1).reshape(HID, 1)
    w["mW2"] = np.asarray(inp["mW2"], np.float32)
    w["mb2"] = np.asarray(inp["mb2"], np.float32).reshape(HID, 1)
    w["mW3"] = np.asarray(inp["mW3"], np.float32)
    w["mb3"] = np.asarray(inp["mb3"], np.float32).reshape(HID, 1)
    w["mW4"] = np.asarray(inp["mW4"], np.float32)
    w["mb4"] = np.asarray(inp["mb4"], np.float32).reshape(1, 2)
    return w


_CACHED_NC = None


def _run(inputs, **spmd_kwargs):
    global _CACHED_NC
    if _CACHED_NC is None:
        _CACHED_NC = build_program(NCL)
    nc = _CACHED_NC
    w = prep_weights(inputs)
    x = np.asarray(inputs["x"], np.float32).reshape(B * P)
    in_maps = []
    for core in range(NCORES):
        m = dict(w)
        m["xT"] = x[core * NCL * P:(core + 1) * NCL * P].reshape(1, NCL * P).copy()
        in_maps.append(m)
    res = bass_utils.run_bass_kernel_spmd(nc, in_maps, list(range(NCORES)),
                                          **spmd_kwargs)
    out = np.concatenate([res.results[c]["out"] for c in range(NCORES)], axis=0)
    return out.astype(np.float32), res


def kernel(**inputs):
    return _run(inputs)[0]


if __name__ == "__main__":
    import reference
    inputs = {k: np.asarray(v) for k, v in reference.setup_inputs().items()}
    got = kernel(**inputs)
    print("out shape:", got.shape)
